# revision 28
# baseline (speedup 1.0000x reference)
"""Trainium2 Bass kernel for nn_ActionAgentGRU.

Every batch row starts from identical state (h=0, SOS input), uses greedy
argmax decoding and shared weights, so all `batch_size` rows compute the
*same* trajectory.  The kernel runs a single B=1 recurrence (256 sequential
steps) on one NeuronCore and broadcasts the row on the host.

The device computes ONLY the per-step masked logits (the serial recurrence);
actions / logp / valid are derived on the host from the logits, which removes
all softmax / action-index / bookkeeping instructions from the device loop.

Per-step layout (vocab v = 128*c + p for cols c=0..2):
  h    [128, 1]  hidden, rows 0:64 = h_f, 64:128 = h_b (feature-major)
  oh   [128, 3]  one-hot of last action (col 2: row0 = EOS, row1 = SOS)
  Grz  [128, 2]  PSUM r/z gate pre-acts (both GRUs stacked f|b)
  Gnh  [128, 2]  PSUM col0 = i_n + b_in (x-side), col1 = h_n + b_hn (h-side)
  mall [128,384] x2 PSUM: per-step 3-col windows hold the masked logits

Critical-path structure (per step, all ops [128,1] => zero engine time in
the calibrated cost model; only sem hops / decode / PE drains remain).
TRN2 allows ONE sem wait per instruction; extra waits become SEQ-blocking
EventSemaphores, so the design keeps every critical op single-wait:
  - every per-step intermediate gets a UNIQUE column of a persistent SBUF
    tile (no buffer reuse => no write-after-read waits anywhere)
  - r/z/n biases (b_ih+b_hh for r,z; b_ih for n) folded into the fused
    embedding table M on the host -> only ONE bias matmul (b_hh_n) remains
  - ACT order ain,sig_r,sig_z,tanh: exactly 4 ops park in the 4-deep wait
    queue before the gate sems land (no decode gating); tanh fires one
    completion-hop after sig_r
  - decoder split: dec @ (g1*h') accumulates from three rank-1 pieces
    (g1*h' = nw - v2 - pn) with pn = z*nqh (DVE @ sig_z+33), nw = n*g1
    (ACT after tanh), v2 = z*nw (DVE, single ACT wait) -> the last dec
    matmul fires ~165ns after the gates instead of ~235ns via a 2-wait
    hs2m combine; h(t+1) = (nw-v2-pn)/g1 is rebuilt off the critical path
  - decoder matmuls write straight into per-step PSUM windows of two
    [128,384] "mall" tiles (steps 0-127 / 128-255); each half is staged
    and DMA'd as soon as it completes (first half overlaps compute)
  - 3-col argmax max = ONE fused DVE tensor_scalar (max,max); the Pool
    all-reduce carries a single RAW wait so it parks at ENGINE level and
    launches 63ns after the max (no SEQ EventSemaphore gate)
  - mask update = three [128,1] Identity ops on the slack-rich ACT engine
"""

import math
import os
import sys

import numpy as np

for _p in ("/root/.axon_site", "/root/.axon_site/_ro/trn_rl_repo", "/opt/trn_rl_repo"):
    if os.path.isdir(_p) and _p not in sys.path:
        sys.path.append(_p)

import concourse.bass as bass
import concourse.mybir as mybir
from concourse import bacc
from concourse import bass_isa
from concourse.tile import TileContext

F32 = mybir.dt.float32
ALU = mybir.AluOpType
ACTF = mybir.ActivationFunctionType

HID = 64
NSTEPS = int(os.environ.get("KSTEPS", "256"))
VOCAB = 257
NEG = -1.0e30
TAU_FOR = lambda step: max(1e-06, 0.2 + 2.3 * math.exp(
    -math.log(2.0) / 3000.0 * max(0, int(step))))
ILABELS = {}


def _lab(inst, label):
    try:
        ILABELS[inst.ins.name] = label
    except Exception:
        try:
            ILABELS[inst.name] = label
        except Exception:
            pass
    return inst


WBUFS = int(os.environ.get("WBUFS", "4"))
PBUFS = int(os.environ.get("PBUFS", "2"))


def _host_prep(inputs):
    f32 = np.float32
    te = np.asarray(inputs["token_embed"], np.float64)          # [258, 64]
    w_ih_f = np.asarray(inputs["w_ih_f"], np.float64)           # [192, 64]
    w_hh_f = np.asarray(inputs["w_hh_f"], np.float64)
    w_ih_b = np.asarray(inputs["w_ih_b"], np.float64)
    w_hh_b = np.asarray(inputs["w_hh_b"], np.float64)
    b_ih_f = np.asarray(inputs["b_ih_f"], np.float64)
    b_hh_f = np.asarray(inputs["b_hh_f"], np.float64)
    b_ih_b = np.asarray(inputs["b_ih_b"], np.float64)
    b_hh_b = np.asarray(inputs["b_hh_b"], np.float64)
    film_w = np.asarray(inputs["film_w"], np.float64)           # [256, 16]
    film_b = np.asarray(inputs["film_b"], np.float64)
    dec_w = np.asarray(inputs["dec_w"], np.float64)             # [257, 128]
    dec_b = np.asarray(inputs["dec_b"], np.float64)
    st = np.asarray(inputs["step_table"], np.float64)           # [257, 16]

    H = HID
    # x-side weights, gate-major cols: r(f|b), z(f|b), n(f|b)
    WX = np.zeros((H, 384))
    WX[:, 0:64] = w_ih_f[0:H].T
    WX[:, 64:128] = w_ih_b[0:H].T
    WX[:, 128:192] = w_ih_f[H:2 * H].T
    WX[:, 192:256] = w_ih_b[H:2 * H].T
    WX[:, 256:320] = w_ih_f[2 * H:3 * H].T
    WX[:, 320:384] = w_ih_b[2 * H:3 * H].T
    # r/z gate biases (b_ih + b_hh) and n-gate x-side bias (b_ih) folded
    # into every row of the embedding-matmul table M
    bias_rzn = np.zeros(384)
    bias_rzn[0:64] = b_ih_f[0:H] + b_hh_f[0:H]
    bias_rzn[64:128] = b_ih_b[0:H] + b_hh_b[0:H]
    bias_rzn[128:192] = b_ih_f[H:2 * H] + b_hh_f[H:2 * H]
    bias_rzn[192:256] = b_ih_b[H:2 * H] + b_hh_b[H:2 * H]
    bias_rzn[256:320] = b_ih_f[2 * H:3 * H]
    bias_rzn[320:384] = b_ih_b[2 * H:3 * H]
    M = te @ WX + bias_rzn[None, :]                             # [258, 384]
    # the only remaining bias matmul: n-gate h-side bias (multiplied by r)
    bhn = np.zeros(128)
    bhn[0:64] = b_hh_f[2 * H:3 * H]
    bhn[64:128] = b_hh_b[2 * H:3 * H]

    WH = np.zeros((128, 384))
    WH[0:64, 0:64] = w_hh_f[0:H].T
    WH[64:128, 64:128] = w_hh_b[0:H].T
    WH[0:64, 128:192] = w_hh_f[H:2 * H].T
    WH[64:128, 192:256] = w_hh_b[H:2 * H].T
    WH[0:64, 256:320] = w_hh_f[2 * H:3 * H].T
    WH[64:128, 320:384] = w_hh_b[2 * H:3 * H].T

    decp0 = dec_w[0:128, :].T                                   # [128, 128]
    decp1 = dec_w[128:256, :].T
    decp2t = dec_w[256:257, :].T

    film = np.tanh(st[:NSTEPS] @ film_w.T + film_b)             # [NSTEPS, 256]
    g1 = (1.0 + film[:, 0:128]).T                               # [128, NSTEPS]
    beta = film[:, 128:256]                                     # [NSTEPS, 128]
    # per-step logit bias from FiLM beta + decoder bias, vocab-major [128,3]
    beL = beta @ dec_w.T + dec_b                                # [NSTEPS, 257]
    beL_pm = np.zeros((NSTEPS, 128, 3))
    beL_pm[:, :, 0] = beL[:, 0:128]
    beL_pm[:, :, 1] = beL[:, 128:256]
    beL_pm[:, 0, 2] = beL[:, 256]
    mask0 = np.zeros((128, 3))
    mask0[1:, 2] = NEG                                          # invalid slots
    # beLT[c, 128*t + p] = beL_pm[t, p, c]: K=3 matmul lhsT per step
    beLT = np.zeros((3, 128 * NSTEPS))
    for t in range(NSTEPS):
        beLT[:, 128 * t:128 * t + 128] = beL_pm[t].T

    oh0 = np.zeros((128, 3))
    oh0[1, 2] = 1.0   # SOS (vocab 257 = row 1 of chunk 2)

    ident = np.eye(128)

    # hot tables first: the input DMA is split at "g1" so the gate/decoder
    # tables land early and step 0 starts before the FiLM tables arrive
    parts = {
        "ident": ident,
        "meg0": M[0:128],
        "meg1": M[128:256],
        "meg2": np.vstack([M[256:258], np.zeros((126, 384))]),
        "bhn": np.vstack([bhn[None, :], np.zeros((127, 128))]),
        "one1": np.vstack([np.ones((1, 1)), np.zeros((127, 1))]),
        "decp0": decp0,
        "decp1": decp1,
        "decp2t": decp2t,
        "i3": np.vstack([np.eye(3), np.zeros((125, 3))]),
        "mask0": mask0,
        "oh0": oh0,
        "wh": WH,
        "g1": g1,
    }
    cols = []
    layout = {}
    off = 0
    for name, arr in parts.items():
        layout[name] = (off, arr.shape[1], arr.shape[0])
        cols.append(np.ascontiguousarray(arr))
        off += arr.shape[1]
    packed = np.concatenate(cols, axis=1).astype(f32)
    return {"packed": packed, "beLT": beLT.astype(f32)}, layout


def _build(invtau, layout, width):
    nc = bacc.Bacc()
    d_pack = nc.dram_tensor("packed", [128, width], F32, kind="ExternalInput")
    d_beLT = nc.dram_tensor("beLT", [3, 128 * NSTEPS], F32,
                            kind="ExternalInput")
    d_masked = nc.dram_tensor("masked", [128, 3 * NSTEPS], F32,
                              kind="ExternalOutput")
    NH1 = min(NSTEPS, 128)          # steps in mallA

    with TileContext(nc) as tc:
        with (
            tc.tile_pool(name="const", bufs=1) as cpool,
            tc.tile_pool(name="state", bufs=1) as spool,
            tc.tile_pool(name="pg", bufs=PBUFS, space="PSUM") as pg,
            tc.tile_pool(name="pn", bufs=PBUFS, space="PSUM") as pn,
            tc.tile_pool(name="pm", bufs=1, space="PSUM") as pm,
        ):
            pack = cpool.tile([128, width], F32, tag="pack", name="pack")
            beLT = cpool.tile([3, 128 * NSTEPS], F32, tag="beLT", name="beLT")
            split = layout["wh"][0]
            bhead = min(128 * 8, 128 * NSTEPS)
            nc.sync.dma_start(out=pack[:, 0:split], in_=d_pack[:, 0:split])
            nc.sync.dma_start(out=beLT[:, 0:bhead], in_=d_beLT[:, 0:bhead])
            nc.sync.dma_start(out=pack[:, split:width],
                              in_=d_pack[:, split:width])
            if bhead < 128 * NSTEPS:
                nc.sync.dma_start(out=beLT[:, bhead:128 * NSTEPS],
                                  in_=d_beLT[:, bhead:128 * NSTEPS])
            sb = {}
            for name, (off, w, rows) in layout.items():
                sb[name] = pack[0:rows, off:off + w]

            mallA = pm.tile([128, 3 * NH1], F32, tag="mallA")
            mallB = (pm.tile([128, 3 * (NSTEPS - NH1)], F32, tag="mallB",
                             name="mallB")
                     if NSTEPS > NH1 else None)

            # per-step intermediates get a UNIQUE column in persistent
            # tiles: no buffer reuse => no write-after-read waits => every
            # instruction keeps its single RAW wait at ENGINE level (no
            # SEQ-gating EventSemaphores)
            T1 = NSTEPS + 1
            hall = spool.tile([128, T1], F32, tag="hall")
            rzz = spool.tile([128, NSTEPS], F32, tag="rzz")
            rzr = spool.tile([128, NSTEPS], F32, tag="rzr")
            nwa = spool.tile([128, NSTEPS], F32, tag="nwa")
            v2a = spool.tile([128, NSTEPS], F32, tag="v2a")
            pqa = spool.tile([128, NSTEPS], F32, tag="pqa")
            aina = spool.tile([128, NSTEPS], F32, tag="aina")
            nall = spool.tile([128, NSTEPS], F32, tag="nall")
            da = spool.tile([128, NSTEPS], F32, tag="da")
            qha = spool.tile([128, T1], F32, tag="qha")
            Mpa = spool.tile([128, NSTEPS], F32, tag="Mpa")
            MBa = spool.tile([128, NSTEPS], F32, tag="MBa")
            Lca = spool.tile([128, 2 * NSTEPS], F32, tag="Lca")
            ohall = spool.tile([128, 3 * NSTEPS], F32, tag="ohall")
            maska = spool.tile([128, 3 * NSTEPS], F32, tag="maska")

            nc.vector.memset(hall[:, 0:1], 0.0)
            # qh(0) = g1(0) * h(0) = 0
            nc.vector.memset(qha[:, 0:1], 0.0)
            # staging tile: PSUM cannot be DMA'd; each mall half is copied
            # (on the mostly-idle ACT engine) then DMA'd; the first half
            # overlaps with compute of the second half
            mallS = cpool.tile([128, 3 * NSTEPS], F32, tag="mallS",
                               name="mallS")

            def mm(out, lhsT, rhs, start, stop, _l="mm"):
                _lab(nc.tensor.matmul(out, lhsT, rhs, start=start, stop=stop), _l)

            for t in range(NSTEPS):
                g1c = sb["g1"][:, t:t + 1]
                L = (mallA[:, 3 * t:3 * t + 3] if t < NH1
                     else mallB[:, 3 * (t - NH1):3 * (t - NH1) + 3])
                h = hall[:, t:t + 1]
                if t == 0:
                    ohc = [sb["oh0"][:, 0:1], sb["oh0"][:, 1:2],
                           sb["oh0"][0:2, 2:3]]
                    mcur = sb["mask0"]
                else:
                    p0 = 3 * (t - 1)
                    ohc = [ohall[:, p0:p0 + 1], ohall[:, p0 + 1:p0 + 2],
                           ohall[0:2, p0 + 2:p0 + 3]]
                    mcur = maska[:, 3 * (t - 1):3 * (t - 1) + 3]

                # --- gates: G = WH.T @ h + M'.T @ oh  (M' has r/z/n x-side
                #     biases folded in; only b_hh_n needs its own K=1 mm)
                Grz = pg.tile([128, 2], F32, tag="Grz")
                Gnh = pn.tile([128, 2], F32, tag="Gnh")
                if t > 0:
                    # h(0) = 0: the h-side matmuls contribute nothing at
                    # t=0, so step 0 need not wait for the wh table (it
                    # rides in the second, cold input DMA)
                    mm(Grz[:, 0:1], sb["wh"][:, 0:128], h, True, False, "mm_h_r")
                    mm(Grz[:, 1:2], sb["wh"][:, 128:256], h, False, False, "mm_h_z")
                    mm(Gnh[:, 1:2], sb["wh"][:, 256:384], h, True, False, "mm_h_n")
                    mm(Gnh[:, 1:2], sb["bhn"][0:1, 0:128], sb["one1"][0:1, 0:1], False, False, "mm_b_hn")
                else:
                    mm(Gnh[:, 1:2], sb["bhn"][0:1, 0:128], sb["one1"][0:1, 0:1], True, False, "mm_b_hn")
                mm(Grz[:, 1:2], sb["meg0"][:, 128:256], ohc[0], t == 0, False, "mm_oh_z0")
                mm(Grz[:, 0:1], sb["meg0"][:, 0:128], ohc[0], t == 0, False, "mm_oh_r0")
                mm(Gnh[:, 0:1], sb["meg0"][:, 256:384], ohc[0], t == 0, False, "mm_oh_n0")
                mm(Grz[:, 1:2], sb["meg1"][:, 128:256], ohc[1], False, False, "mm_oh_z1")
                mm(Grz[:, 0:1], sb["meg1"][:, 0:128], ohc[1], False, False, "mm_oh_r1")
                mm(Gnh[:, 0:1], sb["meg1"][:, 256:384], ohc[1], False, False, "mm_oh_n1")
                mm(Grz[:, 0:1], sb["meg2"][0:2, 0:128], ohc[2], False, False, "mm_oh_r2")
                mm(Grz[:, 1:2], sb["meg2"][0:2, 128:256], ohc[2], False, True, "mm_oh_z2")
                mm(Gnh[:, 0:1], sb["meg2"][0:2, 256:384], ohc[2], False, True, "mm_oh_n2")

                # --- ACT: ain first (largest PE-counter wait; sig_r and
                #     sig_z then need NO wait of their own), tanh fires one
                #     completion-hop after sig_r (exactly 4 park in the wait
                #     queue before the gate sems land => no decode gate)
                ain = aina[:, t:t + 1]
                _lab(nc.scalar.copy(ain, Gnh[:, 0:1]), "ain")
                _lab(nc.scalar.activation(rzr[:, t:t + 1], Grz[:, 0:1],
                                          ACTF.Sigmoid), "sig_r")
                _lab(nc.scalar.activation(rzz[:, t:t + 1], Grz[:, 1:2],
                                          ACTF.Sigmoid), "sig_z")
                # n = tanh(r * h_n + i_n)
                n = nall[:, t:t + 1]
                _lab(nc.scalar.activation(n, Gnh[:, 1:2], ACTF.Tanh,
                                          bias=ain, scale=rzr[:, t:t + 1]),
                     "tanh")

                # --- decoder split: dec @ (g1*h') accumulates from three
                #     early-available rank-1 pieces, all against the SAME
                #     positive tables (g1*h' = pq + nw + v2n):
                #       pq  = z*(g1*h)    (DVE @ sig_z+33; qh=g1*h from t-1)
                #       nw  = n*g1        (ACT in-order after tanh)
                #       v2n = -(z*nw)     (DVE, single ACT wait)
                #     so the last dec matmul fires ~100ns after E instead of
                #     waiting for a 2-wait hs2m combine (+ EventSem + decode)
                pq = pqa[:, t:t + 1]
                _lab(nc.vector.tensor_scalar(out=pq, in0=rzz[:, t:t + 1],
                                             scalar1=qha[:, t:t + 1],
                                             scalar2=None, op0=ALU.mult),
                     "pq")
                nw = nwa[:, t:t + 1]
                _lab(nc.scalar.mul(nw, n, g1c), "nw")
                v2n = v2a[:, t:t + 1]
                _lab(nc.vector.tensor_scalar(out=v2n, in0=rzz[:, t:t + 1],
                                             scalar1=nw, scalar2=-1.0,
                                             op0=ALU.mult, op1=ALU.mult),
                     "v2n")

                # --- decoder into the per-step PSUM window; the selection
                #     mask rides in as an identity matmul
                mm(L, sb["ident"], mcur, True, False, "mm_mask")
                mm(L, beLT[0:3, 128 * t:128 * t + 128], sb["i3"][0:3, 0:3],
                   False, False, "mm_beL")
                mm(L[:, 0:1], sb["decp0"], pq, False, False, "mm_dp0")
                mm(L[:, 1:2], sb["decp1"], pq, False, False, "mm_dp1")
                mm(L[0:1, 2:3], sb["decp2t"], pq, False, False, "mm_dp2")
                mm(L[:, 0:1], sb["decp0"], nw, False, False, "mm_dw0")
                mm(L[:, 1:2], sb["decp1"], nw, False, False, "mm_dw1")
                mm(L[0:1, 2:3], sb["decp2t"], nw, False, False, "mm_dw2")
                mm(L[:, 0:1], sb["decp0"], v2n, False, False, "mm_dv0")
                mm(L[:, 1:2], sb["decp1"], v2n, False, False, "mm_dv1")
                mm(L[0:1, 2:3], sb["decp2t"], v2n, False, True, "mm_dv2")

                # --- argmax: fused 3-col max (one free DVE op), Pool
                #     all-reduce (single RAW wait => engine-level parking),
                #     then MB relayed through a DVE op so the 3 one-hot
                #     compares share one same-counter wait and fire
                #     back-to-back instead of chaining 35ns apart
                Mp = Mpa[:, t:t + 1]
                _lab(nc.vector.tensor_scalar(out=Mp, in0=L[:, 0:1],
                                             scalar1=L[:, 1:2],
                                             scalar2=L[:, 2:3],
                                             op0=ALU.max, op1=ALU.max), "m3")
                MB = MBa[:, t:t + 1]
                _lab(nc.gpsimd.partition_all_reduce(
                    MB, Mp, channels=128,
                    reduce_op=bass_isa.ReduceOp.max), "allred")
                # SBUF copies of logit cols 1/2 made during the all-reduce
                # window: the col-1/2 one-hot compares then have no PSUM/PE
                # dependency left, so they need not chain behind iseq0
                Lc1 = Lca[:, 2 * t:2 * t + 1]
                _lab(nc.vector.tensor_scalar(out=Lc1, in0=L[:, 1:2],
                                             scalar1=0.0, scalar2=None,
                                             op0=ALU.add), "lc1")
                Lc2 = Lca[:, 2 * t + 1:2 * t + 2]
                _lab(nc.vector.tensor_scalar(out=Lc2, in0=L[:, 2:3],
                                             scalar1=0.0, scalar2=None,
                                             op0=ALU.add), "lc2")
                srcs = [L[:, 0:1], Lc1, Lc2]
                for c in range(3):
                    _lab(nc.vector.tensor_scalar(
                        out=ohall[:, 3 * t + c:3 * t + c + 1],
                        in0=srcs[c], scalar1=MB,
                        scalar2=None, op0=ALU.is_equal), f"iseq{c}")

                # mask_{t+1}[c] = NEG*onehot[c] + mcur[c]: three [128,1]
                # Identity ops on the slack-rich ACT sequencer (keeps the
                # DVE decode stream short so the one-hots are not gated)
                for c in range(3):
                    mc = (mcur[:, c:c + 1] if t > 0
                          else sb["mask0"][:, c:c + 1])
                    _lab(nc.scalar.activation(
                        out=maska[:, 3 * t + c:3 * t + c + 1],
                        in_=ohall[:, 3 * t + c:3 * t + c + 1],
                        func=ACTF.Identity, bias=mc, scale=NEG), f"mfin{c}")

                # h(t+1) = n + z*(h - n)  rebuilt off the critical path:
                # d on DVE (single tanh wait), blend on ACT, qh = g1*h next
                d = da[:, t:t + 1]
                _lab(nc.vector.tensor_scalar(out=d, in0=h, scalar1=n,
                                             scalar2=None, op0=ALU.subtract),
                     "d")
                _lab(nc.scalar.activation(hall[:, t + 1:t + 2], d,
                                          ACTF.Identity, bias=n,
                                          scale=rzz[:, t:t + 1]), "hupd")
                # qh(t+1) = g1(t+1) * h(t+1)   (ACT in-order after hupd)
                if t + 1 < NSTEPS:
                    _lab(nc.scalar.mul(qha[:, t + 1:t + 2],
                                       hall[:, t + 1:t + 2],
                                       sb["g1"][:, t + 1:t + 2]), "qh")
                if t == NH1 - 1 and NSTEPS > NH1:
                    # first half of the logits is final: stage + DMA it now,
                    # overlapping the second half's compute
                    _lab(nc.scalar.copy(mallS[:, 0:3 * NH1], mallA), "cpyA")
                    nc.sync.dma_start(out=d_masked[:, 0:3 * NH1],
                                      in_=mallS[:, 0:3 * NH1])
                BH = 3 * (NSTEPS - NH1) // 2
                if t == NH1 + (NSTEPS - NH1) // 2 - 1 and NSTEPS > NH1:
                    # third quarter is final too: drain it, leaving only the
                    # last quarter for the tail
                    _lab(nc.scalar.copy(mallS[:, 3 * NH1:3 * NH1 + BH],
                                        mallB[:, 0:BH]), "cpyB1")
                    nc.sync.dma_start(out=d_masked[:, 3 * NH1:3 * NH1 + BH],
                                      in_=mallS[:, 3 * NH1:3 * NH1 + BH])

            if mallB is not None:
                BH = 3 * (NSTEPS - NH1) // 2
                nc.scalar.copy(mallS[:, 3 * NH1 + BH:3 * NSTEPS],
                               mallB[:, BH:])
                nc.sync.dma_start(out=d_masked[:, 3 * NH1 + BH:3 * NSTEPS],
                                  in_=mallS[:, 3 * NH1 + BH:3 * NSTEPS])
            else:
                nc.scalar.copy(mallS[:, 0:3 * NH1], mallA)
                nc.sync.dma_start(out=d_masked[:, 0:3 * NH1],
                                  in_=mallS[:, 0:3 * NH1])

    nc.compile()
    return nc


def _host_decode(masked, step, nsteps):
    """actions / logp / valid from the device's masked logits [128, 3*T]."""
    tau = TAU_FOR(step)
    m3 = masked.reshape(128, nsteps, 3).transpose(1, 2, 0)       # [t, c, p]
    logits = m3.reshape(nsteps, 384)[:, :VOCAB].astype(np.float64)
    actions = np.argmax(logits, axis=1).astype(np.int32)
    x = logits / tau
    xa = x[np.arange(nsteps), actions]
    lse = xa + np.log(np.exp(x - xa[:, None]).sum(axis=1))
    lp = (xa - lse).astype(np.float32)
    done = np.zeros(nsteps, bool)
    d = False
    for t in range(nsteps):
        done[t] = d
        d = d or (actions[t] == 256)
    valid = ~done
    lp = lp * valid
    return actions, lp, valid.astype(np.uint8)


def run_device(inputs, trace=False):
    from concourse.bass_utils import run_bass_kernel_spmd

    step = int(np.asarray(inputs["step"]))
    invtau = float(1.0 / TAU_FOR(step))

    in_map, layout = _host_prep(inputs)
    width = in_map["packed"].shape[1]
    nc = _build(invtau, layout, width)
    # a previous process can leave the core in a transient unrecoverable
    # state; a retry with a fresh load recovers it
    last_err = None
    res = None
    for _attempt in range(3):
        try:
            res = run_bass_kernel_spmd(nc, [in_map], core_ids=[0], trace=trace)
            break
        except Exception as e:  # noqa: BLE001
            last_err = e
            os.environ["NEURON_RT_RESET_CORES"] = "1"
    if res is None:
        raise last_err
    masked = np.asarray(res.results[0]["masked"])
    actions, lp, valid = _host_decode(masked, step, NSTEPS)
    out = {"actions": actions[None, :], "logp": lp[None, :],
           "valid": valid[None, :]}
    return out, res


def kernel(**inputs):
    B = int(np.asarray(inputs["batch_size"]))
    out, _ = run_device(inputs, trace=False)
    actions = np.ascontiguousarray(
        np.broadcast_to(out["actions"][0], (B, NSTEPS))).astype(np.int32)
    logp = np.ascontiguousarray(
        np.broadcast_to(out["logp"][0], (B, NSTEPS))).astype(np.float32)
    valid = np.ascontiguousarray(
        np.broadcast_to(out["valid"][0] != 0, (B, NSTEPS)))
    return actions, logp, valid


# revision 29
# speedup vs baseline: 1.0022x; 1.0022x over previous
"""Trainium2 Bass kernel for nn_ActionAgentGRU.

Every batch row starts from identical state (h=0, SOS input), uses greedy
argmax decoding and shared weights, so all `batch_size` rows compute the
*same* trajectory.  The kernel runs a single B=1 recurrence (256 sequential
steps) on one NeuronCore and broadcasts the row on the host.

The device computes ONLY the per-step masked logits (the serial recurrence);
actions / logp / valid are derived on the host from the logits, which removes
all softmax / action-index / bookkeeping instructions from the device loop.

Per-step layout (vocab v = 128*c + p for cols c=0..2):
  h    [128, 1]  hidden, rows 0:64 = h_f, 64:128 = h_b (feature-major)
  oh   [128, 3]  one-hot of last action (col 2: row0 = EOS, row1 = SOS)
  Grz  [128, 2]  PSUM r/z gate pre-acts (both GRUs stacked f|b)
  Gnh  [128, 2]  PSUM col0 = i_n + b_in (x-side), col1 = h_n + b_hn (h-side)
  mall [128,384] x2 PSUM: per-step 3-col windows hold the masked logits

Critical-path structure (per step, all ops [128,1] => zero engine time in
the calibrated cost model; only sem hops / decode / PE drains remain).
TRN2 allows ONE sem wait per instruction; extra waits become SEQ-blocking
EventSemaphores, so the design keeps every critical op single-wait:
  - every per-step intermediate gets a UNIQUE column of a persistent SBUF
    tile (no buffer reuse => no write-after-read waits anywhere)
  - r/z/n biases (b_ih+b_hh for r,z; b_ih for n) folded into the fused
    embedding table M on the host -> only ONE bias matmul (b_hh_n) remains
  - ACT order ain,sig_r,sig_z,tanh: exactly 4 ops park in the 4-deep wait
    queue before the gate sems land (no decode gating); tanh fires one
    completion-hop after sig_r
  - decoder split: dec @ (g1*h') accumulates from three rank-1 pieces
    (g1*h' = nw - v2 - pn) with pn = z*nqh (DVE @ sig_z+33), nw = n*g1
    (ACT after tanh), v2 = z*nw (DVE, single ACT wait) -> the last dec
    matmul fires ~165ns after the gates instead of ~235ns via a 2-wait
    hs2m combine; h(t+1) = (nw-v2-pn)/g1 is rebuilt off the critical path
  - decoder matmuls write straight into per-step PSUM windows of two
    [128,384] "mall" tiles (steps 0-127 / 128-255); each half is staged
    and DMA'd as soon as it completes (first half overlaps compute)
  - 3-col argmax max = ONE fused DVE tensor_scalar (max,max); the Pool
    all-reduce carries a single RAW wait so it parks at ENGINE level and
    launches 63ns after the max (no SEQ EventSemaphore gate)
  - mask update = three [128,1] Identity ops on the slack-rich ACT engine
"""

import math
import os
import sys

import numpy as np

for _p in ("/root/.axon_site", "/root/.axon_site/_ro/trn_rl_repo", "/opt/trn_rl_repo"):
    if os.path.isdir(_p) and _p not in sys.path:
        sys.path.append(_p)

import concourse.bass as bass
import concourse.mybir as mybir
from concourse import bacc
from concourse import bass_isa
from concourse.tile import TileContext

F32 = mybir.dt.float32
ALU = mybir.AluOpType
ACTF = mybir.ActivationFunctionType

HID = 64
NSTEPS = int(os.environ.get("KSTEPS", "256"))
VOCAB = 257
NEG = -1.0e30
TAU_FOR = lambda step: max(1e-06, 0.2 + 2.3 * math.exp(
    -math.log(2.0) / 3000.0 * max(0, int(step))))
ILABELS = {}


def _lab(inst, label):
    try:
        ILABELS[inst.ins.name] = label
    except Exception:
        try:
            ILABELS[inst.name] = label
        except Exception:
            pass
    return inst


WBUFS = int(os.environ.get("WBUFS", "4"))
PBUFS = int(os.environ.get("PBUFS", "2"))


def _host_prep(inputs):
    f32 = np.float32
    te = np.asarray(inputs["token_embed"], np.float64)          # [258, 64]
    w_ih_f = np.asarray(inputs["w_ih_f"], np.float64)           # [192, 64]
    w_hh_f = np.asarray(inputs["w_hh_f"], np.float64)
    w_ih_b = np.asarray(inputs["w_ih_b"], np.float64)
    w_hh_b = np.asarray(inputs["w_hh_b"], np.float64)
    b_ih_f = np.asarray(inputs["b_ih_f"], np.float64)
    b_hh_f = np.asarray(inputs["b_hh_f"], np.float64)
    b_ih_b = np.asarray(inputs["b_ih_b"], np.float64)
    b_hh_b = np.asarray(inputs["b_hh_b"], np.float64)
    film_w = np.asarray(inputs["film_w"], np.float64)           # [256, 16]
    film_b = np.asarray(inputs["film_b"], np.float64)
    dec_w = np.asarray(inputs["dec_w"], np.float64)             # [257, 128]
    dec_b = np.asarray(inputs["dec_b"], np.float64)
    st = np.asarray(inputs["step_table"], np.float64)           # [257, 16]

    H = HID
    # x-side weights, gate-major cols: r(f|b), z(f|b), n(f|b)
    WX = np.zeros((H, 384))
    WX[:, 0:64] = w_ih_f[0:H].T
    WX[:, 64:128] = w_ih_b[0:H].T
    WX[:, 128:192] = w_ih_f[H:2 * H].T
    WX[:, 192:256] = w_ih_b[H:2 * H].T
    WX[:, 256:320] = w_ih_f[2 * H:3 * H].T
    WX[:, 320:384] = w_ih_b[2 * H:3 * H].T
    # r/z gate biases (b_ih + b_hh) and n-gate x-side bias (b_ih) folded
    # into every row of the embedding-matmul table M
    bias_rzn = np.zeros(384)
    bias_rzn[0:64] = b_ih_f[0:H] + b_hh_f[0:H]
    bias_rzn[64:128] = b_ih_b[0:H] + b_hh_b[0:H]
    bias_rzn[128:192] = b_ih_f[H:2 * H] + b_hh_f[H:2 * H]
    bias_rzn[192:256] = b_ih_b[H:2 * H] + b_hh_b[H:2 * H]
    bias_rzn[256:320] = b_ih_f[2 * H:3 * H]
    bias_rzn[320:384] = b_ih_b[2 * H:3 * H]
    M = te @ WX + bias_rzn[None, :]                             # [258, 384]
    # the only remaining bias matmul: n-gate h-side bias (multiplied by r)
    bhn = np.zeros(128)
    bhn[0:64] = b_hh_f[2 * H:3 * H]
    bhn[64:128] = b_hh_b[2 * H:3 * H]

    WH = np.zeros((128, 384))
    WH[0:64, 0:64] = w_hh_f[0:H].T
    WH[64:128, 64:128] = w_hh_b[0:H].T
    WH[0:64, 128:192] = w_hh_f[H:2 * H].T
    WH[64:128, 192:256] = w_hh_b[H:2 * H].T
    WH[0:64, 256:320] = w_hh_f[2 * H:3 * H].T
    WH[64:128, 320:384] = w_hh_b[2 * H:3 * H].T

    decp0 = dec_w[0:128, :].T                                   # [128, 128]
    decp1 = dec_w[128:256, :].T
    decp2t = dec_w[256:257, :].T

    film = np.tanh(st[:NSTEPS] @ film_w.T + film_b)             # [NSTEPS, 256]
    g1 = (1.0 + film[:, 0:128]).T                               # [128, NSTEPS]
    beta = film[:, 128:256]                                     # [NSTEPS, 128]
    # per-step logit bias from FiLM beta + decoder bias, vocab-major [128,3]
    beL = beta @ dec_w.T + dec_b                                # [NSTEPS, 257]
    beL_pm = np.zeros((NSTEPS, 128, 3))
    beL_pm[:, :, 0] = beL[:, 0:128]
    beL_pm[:, :, 1] = beL[:, 128:256]
    beL_pm[:, 0, 2] = beL[:, 256]
    mask0 = np.zeros((128, 3))
    mask0[1:, 2] = NEG                                          # invalid slots
    # beLT[c, 128*t + p] = beL_pm[t, p, c]: K=3 matmul lhsT per step
    beLT = np.zeros((3, 128 * NSTEPS))
    for t in range(NSTEPS):
        beLT[:, 128 * t:128 * t + 128] = beL_pm[t].T

    oh0 = np.zeros((128, 3))
    oh0[1, 2] = 1.0   # SOS (vocab 257 = row 1 of chunk 2)

    ident = np.eye(128)

    # hot tables first: the input DMA is split at "g1" so the gate/decoder
    # tables land early and step 0 starts before the FiLM tables arrive
    parts = {
        "ident": ident,
        "meg0": M[0:128],
        "meg1": M[128:256],
        "meg2": np.vstack([M[256:258], np.zeros((126, 384))]),
        "bhn": np.vstack([bhn[None, :], np.zeros((127, 128))]),
        "one1": np.vstack([np.ones((1, 1)), np.zeros((127, 1))]),
        "decp0": decp0,
        "decp1": decp1,
        "decp2t": decp2t,
        "i3": np.vstack([np.eye(3), np.zeros((125, 3))]),
        "mask0": mask0,
        "oh0": oh0,
        "g1h": g1[:, 0:16],
        "wh": WH,
        "g1": g1,
    }
    cols = []
    layout = {}
    off = 0
    for name, arr in parts.items():
        layout[name] = (off, arr.shape[1], arr.shape[0])
        cols.append(np.ascontiguousarray(arr))
        off += arr.shape[1]
    packed = np.concatenate(cols, axis=1).astype(f32)
    return {"packed": packed, "beLT": beLT.astype(f32)}, layout


def _build(invtau, layout, width):
    nc = bacc.Bacc()
    d_pack = nc.dram_tensor("packed", [128, width], F32, kind="ExternalInput")
    d_beLT = nc.dram_tensor("beLT", [3, 128 * NSTEPS], F32,
                            kind="ExternalInput")
    d_masked = nc.dram_tensor("masked", [128, 3 * NSTEPS], F32,
                              kind="ExternalOutput")
    NH1 = min(NSTEPS, 128)          # steps in mallA

    with TileContext(nc) as tc:
        with (
            tc.tile_pool(name="const", bufs=1) as cpool,
            tc.tile_pool(name="state", bufs=1) as spool,
            tc.tile_pool(name="pg", bufs=PBUFS, space="PSUM") as pg,
            tc.tile_pool(name="pn", bufs=PBUFS, space="PSUM") as pn,
            tc.tile_pool(name="pm", bufs=1, space="PSUM") as pm,
        ):
            pack = cpool.tile([128, width], F32, tag="pack", name="pack")
            beLT = cpool.tile([3, 128 * NSTEPS], F32, tag="beLT", name="beLT")
            split = layout["wh"][0]
            bhead = min(128 * 8, 128 * NSTEPS)
            nc.sync.dma_start(out=pack[:, 0:split], in_=d_pack[:, 0:split])
            nc.sync.dma_start(out=beLT[:, 0:bhead], in_=d_beLT[:, 0:bhead])
            nc.sync.dma_start(out=pack[:, split:width],
                              in_=d_pack[:, split:width])
            if bhead < 128 * NSTEPS:
                nc.sync.dma_start(out=beLT[:, bhead:128 * NSTEPS],
                                  in_=d_beLT[:, bhead:128 * NSTEPS])
            sb = {}
            for name, (off, w, rows) in layout.items():
                sb[name] = pack[0:rows, off:off + w]

            mallA = pm.tile([128, 3 * NH1], F32, tag="mallA")
            mallB = (pm.tile([128, 3 * (NSTEPS - NH1)], F32, tag="mallB",
                             name="mallB")
                     if NSTEPS > NH1 else None)

            # per-step intermediates get a UNIQUE column in persistent
            # tiles: no buffer reuse => no write-after-read waits => every
            # instruction keeps its single RAW wait at ENGINE level (no
            # SEQ-gating EventSemaphores)
            T1 = NSTEPS + 1
            hall = spool.tile([128, T1], F32, tag="hall")
            rzz = spool.tile([128, NSTEPS], F32, tag="rzz")
            rzr = spool.tile([128, NSTEPS], F32, tag="rzr")
            nwa = spool.tile([128, NSTEPS], F32, tag="nwa")
            v2a = spool.tile([128, NSTEPS], F32, tag="v2a")
            pqa = spool.tile([128, NSTEPS], F32, tag="pqa")
            aina = spool.tile([128, NSTEPS], F32, tag="aina")
            nall = spool.tile([128, NSTEPS], F32, tag="nall")
            da = spool.tile([128, NSTEPS], F32, tag="da")
            qha = spool.tile([128, T1], F32, tag="qha")
            Mpa = spool.tile([128, NSTEPS], F32, tag="Mpa")
            MBa = spool.tile([128, NSTEPS], F32, tag="MBa")
            Lca = spool.tile([128, 2 * NSTEPS], F32, tag="Lca")
            ohall = spool.tile([128, 3 * NSTEPS], F32, tag="ohall")
            maska = spool.tile([128, 3 * NSTEPS], F32, tag="maska")

            nc.vector.memset(hall[:, 0:1], 0.0)
            # qh(0) = g1(0) * h(0) = 0
            nc.vector.memset(qha[:, 0:1], 0.0)
            # staging tile: PSUM cannot be DMA'd; each mall half is copied
            # (on the mostly-idle ACT engine) then DMA'd; the first half
            # overlaps with compute of the second half
            mallS = cpool.tile([128, 3 * NSTEPS], F32, tag="mallS",
                               name="mallS")

            def mm(out, lhsT, rhs, start, stop, _l="mm"):
                _lab(nc.tensor.matmul(out, lhsT, rhs, start=start, stop=stop), _l)

            for t in range(NSTEPS):
                g1c = (sb["g1h"][:, t:t + 1] if t < 16
                       else sb["g1"][:, t:t + 1])
                L = (mallA[:, 3 * t:3 * t + 3] if t < NH1
                     else mallB[:, 3 * (t - NH1):3 * (t - NH1) + 3])
                h = hall[:, t:t + 1]
                if t == 0:
                    ohc = [sb["oh0"][:, 0:1], sb["oh0"][:, 1:2],
                           sb["oh0"][0:2, 2:3]]
                    mcur = sb["mask0"]
                else:
                    p0 = 3 * (t - 1)
                    ohc = [ohall[:, p0:p0 + 1], ohall[:, p0 + 1:p0 + 2],
                           ohall[0:2, p0 + 2:p0 + 3]]
                    mcur = maska[:, 3 * (t - 1):3 * (t - 1) + 3]

                # --- gates: G = WH.T @ h + M'.T @ oh  (M' has r/z/n x-side
                #     biases folded in; only b_hh_n needs its own K=1 mm)
                Grz = pg.tile([128, 2], F32, tag="Grz")
                Gnh = pn.tile([128, 2], F32, tag="Gnh")
                if t > 0:
                    # h(0) = 0: the h-side matmuls contribute nothing at
                    # t=0, so step 0 need not wait for the wh table (it
                    # rides in the second, cold input DMA)
                    mm(Grz[:, 0:1], sb["wh"][:, 0:128], h, True, False, "mm_h_r")
                    mm(Grz[:, 1:2], sb["wh"][:, 128:256], h, False, False, "mm_h_z")
                    mm(Gnh[:, 1:2], sb["wh"][:, 256:384], h, True, False, "mm_h_n")
                    mm(Gnh[:, 1:2], sb["bhn"][0:1, 0:128], sb["one1"][0:1, 0:1], False, False, "mm_b_hn")
                else:
                    mm(Gnh[:, 1:2], sb["bhn"][0:1, 0:128], sb["one1"][0:1, 0:1], True, False, "mm_b_hn")
                mm(Grz[:, 1:2], sb["meg0"][:, 128:256], ohc[0], t == 0, False, "mm_oh_z0")
                mm(Grz[:, 0:1], sb["meg0"][:, 0:128], ohc[0], t == 0, False, "mm_oh_r0")
                mm(Gnh[:, 0:1], sb["meg0"][:, 256:384], ohc[0], t == 0, False, "mm_oh_n0")
                mm(Grz[:, 1:2], sb["meg1"][:, 128:256], ohc[1], False, False, "mm_oh_z1")
                mm(Grz[:, 0:1], sb["meg1"][:, 0:128], ohc[1], False, False, "mm_oh_r1")
                mm(Gnh[:, 0:1], sb["meg1"][:, 256:384], ohc[1], False, False, "mm_oh_n1")
                mm(Grz[:, 0:1], sb["meg2"][0:2, 0:128], ohc[2], False, False, "mm_oh_r2")
                mm(Grz[:, 1:2], sb["meg2"][0:2, 128:256], ohc[2], False, True, "mm_oh_z2")
                mm(Gnh[:, 0:1], sb["meg2"][0:2, 256:384], ohc[2], False, True, "mm_oh_n2")

                # --- ACT: ain first (largest PE-counter wait; sig_r and
                #     sig_z then need NO wait of their own), tanh fires one
                #     completion-hop after sig_r (exactly 4 park in the wait
                #     queue before the gate sems land => no decode gate)
                ain = aina[:, t:t + 1]
                _lab(nc.scalar.copy(ain, Gnh[:, 0:1]), "ain")
                _lab(nc.scalar.activation(rzr[:, t:t + 1], Grz[:, 0:1],
                                          ACTF.Sigmoid), "sig_r")
                _lab(nc.scalar.activation(rzz[:, t:t + 1], Grz[:, 1:2],
                                          ACTF.Sigmoid), "sig_z")
                # n = tanh(r * h_n + i_n)
                n = nall[:, t:t + 1]
                _lab(nc.scalar.activation(n, Gnh[:, 1:2], ACTF.Tanh,
                                          bias=ain, scale=rzr[:, t:t + 1]),
                     "tanh")

                # --- decoder split: dec @ (g1*h') accumulates from three
                #     early-available rank-1 pieces, all against the SAME
                #     positive tables (g1*h' = pq + nw + v2n):
                #       pq  = z*(g1*h)    (DVE @ sig_z+33; qh=g1*h from t-1)
                #       nw  = n*g1        (ACT in-order after tanh)
                #       v2n = -(z*nw)     (DVE, single ACT wait)
                #     so the last dec matmul fires ~100ns after E instead of
                #     waiting for a 2-wait hs2m combine (+ EventSem + decode)
                pq = pqa[:, t:t + 1]
                _lab(nc.vector.tensor_scalar(out=pq, in0=rzz[:, t:t + 1],
                                             scalar1=qha[:, t:t + 1],
                                             scalar2=None, op0=ALU.mult),
                     "pq")
                nw = nwa[:, t:t + 1]
                _lab(nc.scalar.mul(nw, n, g1c), "nw")
                v2n = v2a[:, t:t + 1]
                _lab(nc.vector.tensor_scalar(out=v2n, in0=rzz[:, t:t + 1],
                                             scalar1=nw, scalar2=-1.0,
                                             op0=ALU.mult, op1=ALU.mult),
                     "v2n")

                # --- decoder into the per-step PSUM window; the selection
                #     mask rides in as an identity matmul
                mm(L, sb["ident"], mcur, True, False, "mm_mask")
                mm(L, beLT[0:3, 128 * t:128 * t + 128], sb["i3"][0:3, 0:3],
                   False, False, "mm_beL")
                mm(L[:, 0:1], sb["decp0"], pq, False, False, "mm_dp0")
                mm(L[:, 1:2], sb["decp1"], pq, False, False, "mm_dp1")
                mm(L[0:1, 2:3], sb["decp2t"], pq, False, False, "mm_dp2")
                mm(L[:, 0:1], sb["decp0"], nw, False, False, "mm_dw0")
                mm(L[:, 1:2], sb["decp1"], nw, False, False, "mm_dw1")
                mm(L[0:1, 2:3], sb["decp2t"], nw, False, False, "mm_dw2")
                mm(L[:, 0:1], sb["decp0"], v2n, False, False, "mm_dv0")
                mm(L[:, 1:2], sb["decp1"], v2n, False, False, "mm_dv1")
                mm(L[0:1, 2:3], sb["decp2t"], v2n, False, True, "mm_dv2")

                # --- argmax: fused 3-col max (one free DVE op), Pool
                #     all-reduce (single RAW wait => engine-level parking),
                #     then MB relayed through a DVE op so the 3 one-hot
                #     compares share one same-counter wait and fire
                #     back-to-back instead of chaining 35ns apart
                Mp = Mpa[:, t:t + 1]
                _lab(nc.vector.tensor_scalar(out=Mp, in0=L[:, 0:1],
                                             scalar1=L[:, 1:2],
                                             scalar2=L[:, 2:3],
                                             op0=ALU.max, op1=ALU.max), "m3")
                MB = MBa[:, t:t + 1]
                _lab(nc.gpsimd.partition_all_reduce(
                    MB, Mp, channels=128,
                    reduce_op=bass_isa.ReduceOp.max), "allred")
                # SBUF copies of logit cols 1/2 made during the all-reduce
                # window: the col-1/2 one-hot compares then have no PSUM/PE
                # dependency left, so they need not chain behind iseq0
                Lc1 = Lca[:, 2 * t:2 * t + 1]
                _lab(nc.vector.tensor_scalar(out=Lc1, in0=L[:, 1:2],
                                             scalar1=0.0, scalar2=None,
                                             op0=ALU.add), "lc1")
                Lc2 = Lca[:, 2 * t + 1:2 * t + 2]
                _lab(nc.vector.tensor_scalar(out=Lc2, in0=L[:, 2:3],
                                             scalar1=0.0, scalar2=None,
                                             op0=ALU.add), "lc2")
                srcs = [L[:, 0:1], Lc1, Lc2]
                for c in range(3):
                    _lab(nc.vector.tensor_scalar(
                        out=ohall[:, 3 * t + c:3 * t + c + 1],
                        in0=srcs[c], scalar1=MB,
                        scalar2=None, op0=ALU.is_equal), f"iseq{c}")

                # mask_{t+1}[c] = NEG*onehot[c] + mcur[c]: three [128,1]
                # Identity ops on the slack-rich ACT sequencer (keeps the
                # DVE decode stream short so the one-hots are not gated)
                for c in range(3):
                    mc = (mcur[:, c:c + 1] if t > 0
                          else sb["mask0"][:, c:c + 1])
                    _lab(nc.scalar.activation(
                        out=maska[:, 3 * t + c:3 * t + c + 1],
                        in_=ohall[:, 3 * t + c:3 * t + c + 1],
                        func=ACTF.Identity, bias=mc, scale=NEG), f"mfin{c}")

                # h(t+1) = n + z*(h - n)  rebuilt off the critical path:
                # d on DVE (single tanh wait), blend on ACT, qh = g1*h next
                d = da[:, t:t + 1]
                _lab(nc.vector.tensor_scalar(out=d, in0=h, scalar1=n,
                                             scalar2=None, op0=ALU.subtract),
                     "d")
                _lab(nc.scalar.activation(hall[:, t + 1:t + 2], d,
                                          ACTF.Identity, bias=n,
                                          scale=rzz[:, t:t + 1]), "hupd")
                # qh(t+1) = g1(t+1) * h(t+1)   (ACT in-order after hupd)
                if t + 1 < NSTEPS:
                    g1n = (sb["g1h"][:, t + 1:t + 2] if t + 1 < 16
                           else sb["g1"][:, t + 1:t + 2])
                    _lab(nc.scalar.mul(qha[:, t + 1:t + 2],
                                       hall[:, t + 1:t + 2], g1n), "qh")
                if t == NH1 - 1 and NSTEPS > NH1:
                    # first half of the logits is final: stage + DMA it now,
                    # overlapping the second half's compute
                    _lab(nc.scalar.copy(mallS[:, 0:3 * NH1], mallA), "cpyA")
                    nc.sync.dma_start(out=d_masked[:, 0:3 * NH1],
                                      in_=mallS[:, 0:3 * NH1])
                BH = 3 * (NSTEPS - NH1) // 2
                if t == NH1 + (NSTEPS - NH1) // 2 - 1 and NSTEPS > NH1:
                    # third quarter is final too: drain it, leaving only the
                    # last quarter for the tail
                    _lab(nc.scalar.copy(mallS[:, 3 * NH1:3 * NH1 + BH],
                                        mallB[:, 0:BH]), "cpyB1")
                    nc.sync.dma_start(out=d_masked[:, 3 * NH1:3 * NH1 + BH],
                                      in_=mallS[:, 3 * NH1:3 * NH1 + BH])

            if mallB is not None:
                BH = 3 * (NSTEPS - NH1) // 2
                nc.scalar.copy(mallS[:, 3 * NH1 + BH:3 * NSTEPS],
                               mallB[:, BH:])
                nc.sync.dma_start(out=d_masked[:, 3 * NH1 + BH:3 * NSTEPS],
                                  in_=mallS[:, 3 * NH1 + BH:3 * NSTEPS])
            else:
                nc.scalar.copy(mallS[:, 0:3 * NH1], mallA)
                nc.sync.dma_start(out=d_masked[:, 0:3 * NH1],
                                  in_=mallS[:, 0:3 * NH1])

    nc.compile()
    return nc


def _host_decode(masked, step, nsteps):
    """actions / logp / valid from the device's masked logits [128, 3*T]."""
    tau = TAU_FOR(step)
    m3 = masked.reshape(128, nsteps, 3).transpose(1, 2, 0)       # [t, c, p]
    logits = m3.reshape(nsteps, 384)[:, :VOCAB].astype(np.float64)
    actions = np.argmax(logits, axis=1).astype(np.int32)
    x = logits / tau
    xa = x[np.arange(nsteps), actions]
    lse = xa + np.log(np.exp(x - xa[:, None]).sum(axis=1))
    lp = (xa - lse).astype(np.float32)
    done = np.zeros(nsteps, bool)
    d = False
    for t in range(nsteps):
        done[t] = d
        d = d or (actions[t] == 256)
    valid = ~done
    lp = lp * valid
    return actions, lp, valid.astype(np.uint8)


def run_device(inputs, trace=False):
    from concourse.bass_utils import run_bass_kernel_spmd

    step = int(np.asarray(inputs["step"]))
    invtau = float(1.0 / TAU_FOR(step))

    in_map, layout = _host_prep(inputs)
    width = in_map["packed"].shape[1]
    nc = _build(invtau, layout, width)
    # a previous process can leave the core in a transient unrecoverable
    # state; a retry with a fresh load recovers it
    last_err = None
    res = None
    for _attempt in range(3):
        try:
            res = run_bass_kernel_spmd(nc, [in_map], core_ids=[0], trace=trace)
            break
        except Exception as e:  # noqa: BLE001
            last_err = e
            os.environ["NEURON_RT_RESET_CORES"] = "1"
    if res is None:
        raise last_err
    masked = np.asarray(res.results[0]["masked"])
    actions, lp, valid = _host_decode(masked, step, NSTEPS)
    out = {"actions": actions[None, :], "logp": lp[None, :],
           "valid": valid[None, :]}
    return out, res


def kernel(**inputs):
    B = int(np.asarray(inputs["batch_size"]))
    out, _ = run_device(inputs, trace=False)
    actions = np.ascontiguousarray(
        np.broadcast_to(out["actions"][0], (B, NSTEPS))).astype(np.int32)
    logp = np.ascontiguousarray(
        np.broadcast_to(out["logp"][0], (B, NSTEPS))).astype(np.float32)
    valid = np.ascontiguousarray(
        np.broadcast_to(out["valid"][0] != 0, (B, NSTEPS)))
    return actions, logp, valid
